# revision 9
# baseline (speedup 1.0000x reference)
"""Trainium2 Bass kernel for nn_ETypePromptModel: logits = einsum('bpd,cpd->bc').

Equivalent to X @ W.T with X=[B, L*D]=[16384, 256], W=[C, L*D]=[4096, 256].
Data-parallel over B across 8 NeuronCores; label2embed replicated.

bf16 pipeline (tolerance 2e-2; bf16 lands ~0.34%, fp8 measured 3.8% - dead):
  - Host: cast to bf16, pre-transpose to K-major, and pack so bulk DMAs
    land 4-8KB-contiguous per partition: the HWDGE generates descriptors
    at ~18ns each and every [128, x] DMA costs >=128 descriptors, so
    descriptor SIZE (not count) is the only DMA throughput lever.
  - Input loads on the sync ring in consumption order, with W n-half A
    split in two pieces so the stream starts on the first 0.64MB instead
    of waiting for the full 1.125MB: [W-A cols 0:1024 (both k) + x0
    head], [W-A cols 1024:2048], [W n-half B], [full X]. The stream's
    first unit consumes j0,j1 first (k-inner order), so piece 2 has an
    extra ~0.9us to land.
  - Stream is phase-reordered: n-half-A of m-tiles 0..XD-1 runs first
    (fed from the x0 head), so full X is only needed from m-tile XD.
  - bf16 warmup matmuls on a memset tile bridge issue->data so the HAM
    clock gate reaches 8/8 right as the real stream starts. The bridge
    must be GAPLESS (an idle PE gap before the stream keeps the clock
    gated ~4us into the real stream - measured), and the memset must be
    on VECTOR (with it on gpsimd the HAM ramp took 6us, not 2.7us -
    measured).
  - Per (m-tile, n-half): 4 chunk matmuls of 512 cols x 2 k-passes in
    k-INNER order (j0:k0,k1; j1:k0,k1; ...) so each PSUM bank stops as
    early as possible and its cast can chase the stream.
  - PSUM -> SBUF drains as one 1024-wide cast per engine per half
    (Vector even pair, Scalar odd pair); per m-tile one full-row 1MB
    output DMA (8KB descriptors) on the sync ring. Keeping the steady
    state on ONE ring matters: running both HWDGE rings all stream long
    interleaves on the shared 16 SDMA engines and straggles receipts by
    ~8us (measured).
  - Tail: the last TWO m-tiles ship per-half, rows split 64/64 across
    the sync+scalar rings (64-descriptor gens, 1.15us each, in parallel
    on both queues). The end-game drain is cross-core HBM-bound at
    ~300GB/s (all 8 cores flush their last MBs simultaneously), so the
    win comes from making the last 2MB AVAILABLE earlier, not from
    burst bandwidth. Scalar-engine trigger congestion and a 2.3us
    full-row descriptor gen on the scalar queue each cost ~1-2us in
    previous revs (measured).
  - Host: upcast gathered bf16 output to fp32.

PE stream floor: 16 mt x 2 k x 4096 cols = 131072 cycles @ 2.4 GHz = 55 us.
Fixed costs in the graded window: ~6.5us front (tc entry + descriptor gen
+ first-piece load at ~260GB/s ramp), ~7.4us nrt kbin postamble
(sem-reset storm, runtime-injected, unavoidable).
"""

import sys

import ml_dtypes
import numpy as np

sys.path.insert(0, "/opt/trn_rl_repo")

B, C, L, D = 16384, 4096, 2, 128
K = L * D  # 256 contraction
N_CORES = 8
B_LOC = B // N_CORES  # 2048
P = 128
N_TILE = 512  # moving free dim per matmul
M_TILES = B_LOC // P  # 16
KT = K // P  # 2 k-tiles
WH = C // 2  # 2048: w n-half width
NH = WH // N_TILE  # 4 chunks per half
XD = 4  # m-tiles covered by the duplicated X head
X0W = KT * XD * P  # 1024 cols of x0 payload appended to the wxa tile
WHH = WH // 2  # 1024: cols per W-A piece
WAP = KT * WHH  # 2048 payload cols per W-A piece

N_PAIRS = 4  # two-bank PSUM pair tiles (8 banks total)
N_OSB = 6
N_WARM = 10  # warmup matmuls (~430ns each cold) bridging the load window

_CACHE = {}
PROFILE = False
TRACE_ALL_CORES = False
LAST_RESULT = None


def _build():
    import concourse.mybir as mybir
    import concourse.tile as tile
    from concourse import bacc

    f32 = mybir.dt.float32
    bf16 = mybir.dt.bfloat16

    nc = bacc.Bacc(
        "TRN2",
        target_bir_lowering=False,
        debug=False,
        enable_asserts=False,
        num_devices=N_CORES,
    )

    wxa_dram = nc.dram_tensor("wxa", [P, WAP + X0W], bf16, kind="ExternalInput").ap()
    wj23_dram = nc.dram_tensor("wj23", [P, WAP], bf16, kind="ExternalInput").ap()
    w1_dram = nc.dram_tensor("w1", [P, KT, WH], bf16, kind="ExternalInput").ap()
    x_dram = nc.dram_tensor("xt", [P, KT, B_LOC], bf16, kind="ExternalInput").ap()
    out_dram = nc.dram_tensor("out", [B_LOC, C], bf16, kind="ExternalOutput").ap()

    with tile.TileContext(nc) as tc:
        with (
            tc.tile_pool(name="const", bufs=1) as const_pool,
            tc.tile_pool(name="big", bufs=1) as big_pool,
            tc.tile_pool(name="osb", bufs=1) as out_pool,
            tc.tile_pool(name="psm", bufs=1, space="PSUM") as psum_mm,
        ):
            # ---- input loads: consumption order, single sync HWDGE ring ----
            wxa = big_pool.tile([P, WAP + X0W], bf16, name="wxa")
            wj23 = big_pool.tile([P, WAP], bf16, name="wj23")
            wk1 = big_pool.tile([P, KT, WH], bf16, name="wk1")
            xk = big_pool.tile([P, KT, B_LOC], bf16, name="xk")
            nc.sync.dma_start(wxa, wxa_dram)
            nc.sync.dma_start(wj23, wj23_dram)
            nc.sync.dma_start(wk1, w1_dram)
            nc.sync.dma_start(xk, x_dram)

            # ---- PE warmup on a memset tile (HAM un-throttles ~2.7us into
            # the burst, right as the real stream starts) ----
            warm_sb = const_pool.tile([P, P + N_TILE], bf16, name="warm_sb")
            nc.vector.memset(warm_sb, 0.0)

            # ---- manually reused buffers ----
            pairs = [
                psum_mm.tile([P, 2, N_TILE], f32, name=f"pp{i}") for i in range(N_PAIRS)
            ]
            osb = [out_pool.tile([P, C], bf16, name=f"osb{i}") for i in range(N_OSB)]

            for _ in range(N_WARM):
                nc.tensor.matmul(
                    pairs[N_PAIRS - 1][:, 1, :],
                    warm_sb[:, :P],
                    warm_sb[:, P:],
                    start=True,
                    stop=True,
                )

            # ---- main stream, phase-reordered around the input loads ----
            order = [(mt, 0) for mt in range(XD)] + [(mt, 1) for mt in range(XD)]
            for mt in range(XD, M_TILES):
                order += [(mt, 0), (mt, 1)]

            def wslice(h, k, j):
                if h == 0:
                    if j < 2:
                        return wxa[:, k * WHH + j * N_TILE : k * WHH + (j + 1) * N_TILE]
                    return wj23[
                        :, k * WHH + (j - 2) * N_TILE : k * WHH + (j - 1) * N_TILE
                    ]
                return wk1[:, k, j * N_TILE : (j + 1) * N_TILE]

            def xslice(mt, h, k):
                if mt < XD and h == 0:
                    base = WAP + k * XD * P + mt * P
                    return wxa[:, base : base + P]
                return xk[:, k, mt * P : (mt + 1) * P]

            pc = 0
            for mt, h in order:
                out_sb = osb[mt % N_OSB]
                prs = [pairs[(pc + a) % N_PAIRS] for a in range(2)]
                banks = [prs[j // 2][:, j % 2, :] for j in range(NH)]
                pc += 2
                # k-inner order: each PSUM bank stops as early as possible so
                # its cast (and the output DMA behind it) can start sooner
                for j in range(NH):
                    for k in range(KT):
                        nc.tensor.matmul(
                            banks[j],
                            xslice(mt, h, k),
                            wslice(h, k, j),
                            start=(k == 0),
                            stop=(k == KT - 1),
                        )
                off = h * WH
                r0 = mt * P
                # one 1024-wide PSUM->SBUF cast per engine per half
                nc.vector.tensor_copy(
                    out=out_sb[:, off : off + 2 * N_TILE],
                    in_=prs[0].rearrange("p a b -> p (a b)"),
                )
                nc.scalar.copy(
                    out_sb[:, off + 2 * N_TILE : off + 4 * N_TILE],
                    prs[1].rearrange("p a b -> p (a b)"),
                )
                if mt < M_TILES - 2:
                    # steady state: one full-row 1MB DMA per m-tile on the
                    # sync ring (8KB descriptors)
                    if h == 1:
                        nc.sync.dma_start(out_dram[r0 : r0 + P, :], out_sb)
                else:
                    # last two m-tiles: ship each half as soon as it is cast,
                    # rows split 64/64 across the two rings. Small (64-desc)
                    # parallel descriptor gens keep both queues moving; the
                    # end-game drain is cross-core HBM-bound (~300GB/s), so
                    # the win is earlier availability, not burst bandwidth.
                    nc.sync.dma_start(
                        out_dram[r0 : r0 + P // 2, off : off + WH],
                        out_sb[: P // 2, off : off + WH],
                    )
                    nc.scalar.dma_start(
                        out_dram[r0 + P // 2 : r0 + P, off : off + WH],
                        out_sb[P // 2 :, off : off + WH],
                    )

    nc.compile()
    return nc


def kernel(batchs, label2embed):
    global LAST_RESULT
    from concourse.bass_utils import run_bass_kernel_spmd

    if "nc" not in _CACHE:
        _CACHE["nc"] = _build()
    nc = _CACHE["nc"]

    # coerce to numpy up front: harness may pass jax arrays, and host-side
    # jax ops could dispatch to the neuron backend (known crash pitfall)
    batchs = np.asarray(batchs)
    label2embed = np.asarray(label2embed)
    assert batchs.shape == (B, L, D) and label2embed.shape == (C, L, D)
    bf16 = ml_dtypes.bfloat16
    # K-major bf16, packed for 4-8KB/partition DMA rows:
    #   xt: [P, KT, B] (row p = k0-row-p ++ k1-row-p)
    #   w1: [P, KT, WH] (row p = k0-cols ++ k1-cols)
    #   wxa: [P, WAP + X0W] = W-A cols 0:1024 (k0 then k1) ++ per-core x0
    #   wj23: [P, WAP] = W-A cols 1024:2048 (k0 then k1)
    xtf = batchs.reshape(B, K).astype(bf16).T.reshape(KT, P, B)  # [KT, P, B]
    wtf = label2embed.reshape(C, K).astype(bf16).T.reshape(KT, P, C)
    xt = np.ascontiguousarray(xtf.transpose(1, 0, 2))  # [P, KT, B]
    wj01 = wtf[:, :, :WHH].transpose(1, 0, 2).reshape(P, WAP)
    wj23 = np.ascontiguousarray(
        wtf[:, :, WHH:WH].transpose(1, 0, 2).reshape(P, WAP)
    )
    w1 = np.ascontiguousarray(wtf[:, :, WH:].transpose(1, 0, 2))  # [P, KT, WH]
    in_maps = [
        {
            "wxa": np.ascontiguousarray(
                np.concatenate(
                    [wj01, xt[:, :, c * B_LOC : c * B_LOC + XD * P].reshape(P, X0W)],
                    axis=1,
                )
            ),
            "wj23": wj23,
            "w1": w1,
            "xt": np.ascontiguousarray(xt[:, :, c * B_LOC : (c + 1) * B_LOC]),
        }
        for c in range(N_CORES)
    ]
    res = run_bass_kernel_spmd(
        nc,
        in_maps,
        core_ids=list(range(N_CORES)),
        trace=PROFILE,
        trace_cores=list(range(N_CORES)) if (PROFILE and TRACE_ALL_CORES) else None,
    )
    LAST_RESULT = res
    return np.concatenate([r["out"] for r in res.results], axis=0).astype(np.float32)


# revision 11
# speedup vs baseline: 1.0124x; 1.0124x over previous
"""Trainium2 Bass kernel for nn_ETypePromptModel: logits = einsum('bpd,cpd->bc').

Equivalent to X @ W.T with X=[B, L*D]=[16384, 256], W=[C, L*D]=[4096, 256].
Data-parallel over B across 8 NeuronCores; label2embed replicated.

bf16 pipeline (tolerance 2e-2; bf16 lands ~0.34%, fp8 measured 3.8% - dead):
  - Host: cast to bf16, pre-transpose to K-major, and pack so bulk DMAs
    land 4-8KB-contiguous per partition: the HWDGE generates descriptors
    at ~18ns each and every [128, x] DMA costs >=128 descriptors, so
    descriptor SIZE (not count) is the only DMA throughput lever.
  - Input loads on the sync ring in consumption order, with W n-half A
    split in two pieces so the stream starts on the first 0.64MB instead
    of waiting for the full 1.125MB: [W-A cols 0:1024 (both k) + x0
    head], [W-A cols 1024:2048], [W n-half B], [full X]. The stream's
    first unit consumes j0,j1 first (k-inner order), so piece 2 has an
    extra ~0.9us to land.
  - Stream is phase-reordered: n-half-A of m-tiles 0..XD-1 runs first
    (fed from the x0 head), so full X is only needed from m-tile XD.
  - bf16 warmup matmuls on a memset tile bridge issue->data so the HAM
    clock gate reaches 8/8 right as the real stream starts. The bridge
    must be GAPLESS (an idle PE gap before the stream keeps the clock
    gated ~4us into the real stream - measured), and the memset must be
    on VECTOR (with it on gpsimd the HAM ramp took 6us, not 2.7us -
    measured).
  - Per (m-tile, n-half): 4 chunk matmuls of 512 cols x 2 k-passes in
    k-INNER order (j0:k0,k1; j1:k0,k1; ...) so each PSUM bank stops as
    early as possible and its cast can chase the stream.
  - PSUM -> SBUF drains as one 1024-wide cast per engine per half
    (Vector even pair, Scalar odd pair); per m-tile one full-row 1MB
    output DMA (8KB descriptors) on the sync ring. Keeping the steady
    state on ONE ring matters: running both HWDGE rings all stream long
    interleaves on the shared 16 SDMA engines and straggles receipts by
    ~8us (measured).
  - Tail: m14's full-row DMA moves to the (otherwise idle) scalar ring;
    m15 ships per-half, rows split 64/64 across both rings, with the
    final half's casts 512-wide alternating engines. The end-game drain
    is cross-core HBM-bound at ~300GB/s (all 8 cores flush their last
    MBs simultaneously), so the win comes from making the last MBs
    available earlier. Splitting m14 per-half as well was tried and
    measured SLOWER (stream grew 2us) - keep m14 as one full row.
  - Host: upcast gathered bf16 output to fp32.

PE stream floor: 16 mt x 2 k x 4096 cols = 131072 cycles @ 2.4 GHz = 55 us.
Fixed costs in the graded window: ~6.5us front (tc entry + descriptor gen
+ first-piece load at ~260GB/s ramp), ~7.4us nrt kbin postamble
(sem-reset storm, runtime-injected, unavoidable).
"""

import sys

import ml_dtypes
import numpy as np

sys.path.insert(0, "/opt/trn_rl_repo")

B, C, L, D = 16384, 4096, 2, 128
K = L * D  # 256 contraction
N_CORES = 8
B_LOC = B // N_CORES  # 2048
P = 128
N_TILE = 512  # moving free dim per matmul
M_TILES = B_LOC // P  # 16
KT = K // P  # 2 k-tiles
WH = C // 2  # 2048: w n-half width
NH = WH // N_TILE  # 4 chunks per half
XD = 4  # m-tiles covered by the duplicated X head
X0W = KT * XD * P  # 1024 cols of x0 payload appended to the wxa tile
WHH = WH // 2  # 1024: cols per W-A piece
WAP = KT * WHH  # 2048 payload cols per W-A piece

N_PAIRS = 4  # two-bank PSUM pair tiles (8 banks total)
N_OSB = 6
N_WARM = 10  # warmup matmuls (~430ns each cold) bridging the load window

_CACHE = {}
PROFILE = False
TRACE_ALL_CORES = False
LAST_RESULT = None


def _build():
    import concourse.mybir as mybir
    import concourse.tile as tile
    from concourse import bacc

    f32 = mybir.dt.float32
    bf16 = mybir.dt.bfloat16

    nc = bacc.Bacc(
        "TRN2",
        target_bir_lowering=False,
        debug=False,
        enable_asserts=False,
        num_devices=N_CORES,
    )

    wxa_dram = nc.dram_tensor("wxa", [P, WAP + X0W], bf16, kind="ExternalInput").ap()
    wj23_dram = nc.dram_tensor("wj23", [P, WAP], bf16, kind="ExternalInput").ap()
    w1_dram = nc.dram_tensor("w1", [P, KT, WH], bf16, kind="ExternalInput").ap()
    x_dram = nc.dram_tensor("xt", [P, KT, B_LOC], bf16, kind="ExternalInput").ap()
    out_dram = nc.dram_tensor("out", [B_LOC, C], bf16, kind="ExternalOutput").ap()

    with tile.TileContext(nc) as tc:
        with (
            tc.tile_pool(name="const", bufs=1) as const_pool,
            tc.tile_pool(name="big", bufs=1) as big_pool,
            tc.tile_pool(name="osb", bufs=1) as out_pool,
            tc.tile_pool(name="psm", bufs=1, space="PSUM") as psum_mm,
        ):
            # ---- input loads: consumption order, single sync HWDGE ring ----
            wxa = big_pool.tile([P, WAP + X0W], bf16, name="wxa")
            wj23 = big_pool.tile([P, WAP], bf16, name="wj23")
            wk1 = big_pool.tile([P, KT, WH], bf16, name="wk1")
            xk = big_pool.tile([P, KT, B_LOC], bf16, name="xk")
            nc.sync.dma_start(wxa, wxa_dram)
            nc.sync.dma_start(wj23, wj23_dram)
            nc.sync.dma_start(wk1, w1_dram)
            nc.sync.dma_start(xk, x_dram)

            # ---- PE warmup on a memset tile (HAM un-throttles ~2.7us into
            # the burst, right as the real stream starts) ----
            warm_sb = const_pool.tile([P, P + N_TILE], bf16, name="warm_sb")
            nc.vector.memset(warm_sb, 0.0)

            # ---- manually reused buffers ----
            pairs = [
                psum_mm.tile([P, 2, N_TILE], f32, name=f"pp{i}") for i in range(N_PAIRS)
            ]
            osb = [out_pool.tile([P, C], bf16, name=f"osb{i}") for i in range(N_OSB)]

            for _ in range(N_WARM):
                nc.tensor.matmul(
                    pairs[N_PAIRS - 1][:, 1, :],
                    warm_sb[:, :P],
                    warm_sb[:, P:],
                    start=True,
                    stop=True,
                )

            # ---- main stream, phase-reordered around the input loads ----
            order = [(mt, 0) for mt in range(XD)] + [(mt, 1) for mt in range(XD)]
            for mt in range(XD, M_TILES):
                order += [(mt, 0), (mt, 1)]

            def wslice(h, k, j):
                if h == 0:
                    if j < 2:
                        return wxa[:, k * WHH + j * N_TILE : k * WHH + (j + 1) * N_TILE]
                    return wj23[
                        :, k * WHH + (j - 2) * N_TILE : k * WHH + (j - 1) * N_TILE
                    ]
                return wk1[:, k, j * N_TILE : (j + 1) * N_TILE]

            def xslice(mt, h, k):
                if mt < XD and h == 0:
                    base = WAP + k * XD * P + mt * P
                    return wxa[:, base : base + P]
                return xk[:, k, mt * P : (mt + 1) * P]

            pc = 0
            for mt, h in order:
                out_sb = osb[mt % N_OSB]
                prs = [pairs[(pc + a) % N_PAIRS] for a in range(2)]
                banks = [prs[j // 2][:, j % 2, :] for j in range(NH)]
                pc += 2
                # k-inner order: each PSUM bank stops as early as possible so
                # its cast (and the output DMA behind it) can start sooner
                for j in range(NH):
                    for k in range(KT):
                        nc.tensor.matmul(
                            banks[j],
                            xslice(mt, h, k),
                            wslice(h, k, j),
                            start=(k == 0),
                            stop=(k == KT - 1),
                        )
                off = h * WH
                r0 = mt * P
                if mt < M_TILES - 1:
                    # steady state: one 1024-wide PSUM->SBUF cast per engine
                    # per half; one full-row 1MB DMA per m-tile (sync ring,
                    # except m14's which goes on the idle scalar ring so the
                    # sync ring is drained for m15's pieces)
                    nc.vector.tensor_copy(
                        out=out_sb[:, off : off + 2 * N_TILE],
                        in_=prs[0].rearrange("p a b -> p (a b)"),
                    )
                    nc.scalar.copy(
                        out_sb[:, off + 2 * N_TILE : off + 4 * N_TILE],
                        prs[1].rearrange("p a b -> p (a b)"),
                    )
                    if h == 1:
                        ring = nc.scalar if mt == M_TILES - 2 else nc.sync
                        ring.dma_start(out_dram[r0 : r0 + P, :], out_sb)
                else:
                    # m15: ship each half as soon as it is cast, rows split
                    # 64/64 across the two rings (parallel descriptor gen on
                    # drained queues); final half's casts are 512-wide
                    # alternating engines as each PSUM bank stops
                    if h == 0:
                        nc.vector.tensor_copy(
                            out=out_sb[:, off : off + 2 * N_TILE],
                            in_=prs[0].rearrange("p a b -> p (a b)"),
                        )
                        nc.scalar.copy(
                            out_sb[:, off + 2 * N_TILE : off + 4 * N_TILE],
                            prs[1].rearrange("p a b -> p (a b)"),
                        )
                    else:
                        engs = [nc.vector, nc.scalar]
                        for j in range(NH):
                            src = prs[j // 2][:, j % 2, :]
                            dst = out_sb[:, off + j * N_TILE : off + (j + 1) * N_TILE]
                            if j % 2 == 0:
                                engs[0].tensor_copy(out=dst, in_=src)
                            else:
                                engs[1].copy(dst, src)
                    nc.sync.dma_start(
                        out_dram[r0 : r0 + P // 2, off : off + WH],
                        out_sb[: P // 2, off : off + WH],
                    )
                    nc.scalar.dma_start(
                        out_dram[r0 + P // 2 : r0 + P, off : off + WH],
                        out_sb[P // 2 :, off : off + WH],
                    )

    nc.compile()
    return nc


def kernel(batchs, label2embed):
    global LAST_RESULT
    from concourse.bass_utils import run_bass_kernel_spmd

    if "nc" not in _CACHE:
        _CACHE["nc"] = _build()
    nc = _CACHE["nc"]

    # coerce to numpy up front: harness may pass jax arrays, and host-side
    # jax ops could dispatch to the neuron backend (known crash pitfall)
    batchs = np.asarray(batchs)
    label2embed = np.asarray(label2embed)
    assert batchs.shape == (B, L, D) and label2embed.shape == (C, L, D)
    bf16 = ml_dtypes.bfloat16
    # K-major bf16, packed for 4-8KB/partition DMA rows:
    #   xt: [P, KT, B] (row p = k0-row-p ++ k1-row-p)
    #   w1: [P, KT, WH] (row p = k0-cols ++ k1-cols)
    #   wxa: [P, WAP + X0W] = W-A cols 0:1024 (k0 then k1) ++ per-core x0
    #   wj23: [P, WAP] = W-A cols 1024:2048 (k0 then k1)
    xtf = batchs.reshape(B, K).astype(bf16).T.reshape(KT, P, B)  # [KT, P, B]
    wtf = label2embed.reshape(C, K).astype(bf16).T.reshape(KT, P, C)
    xt = np.ascontiguousarray(xtf.transpose(1, 0, 2))  # [P, KT, B]
    wj01 = wtf[:, :, :WHH].transpose(1, 0, 2).reshape(P, WAP)
    wj23 = np.ascontiguousarray(
        wtf[:, :, WHH:WH].transpose(1, 0, 2).reshape(P, WAP)
    )
    w1 = np.ascontiguousarray(wtf[:, :, WH:].transpose(1, 0, 2))  # [P, KT, WH]
    in_maps = [
        {
            "wxa": np.ascontiguousarray(
                np.concatenate(
                    [wj01, xt[:, :, c * B_LOC : c * B_LOC + XD * P].reshape(P, X0W)],
                    axis=1,
                )
            ),
            "wj23": wj23,
            "w1": w1,
            "xt": np.ascontiguousarray(xt[:, :, c * B_LOC : (c + 1) * B_LOC]),
        }
        for c in range(N_CORES)
    ]
    res = run_bass_kernel_spmd(
        nc,
        in_maps,
        core_ids=list(range(N_CORES)),
        trace=PROFILE,
        trace_cores=list(range(N_CORES)) if (PROFILE and TRACE_ALL_CORES) else None,
    )
    LAST_RESULT = res
    return np.concatenate([r["out"] for r in res.results], axis=0).astype(np.float32)
